# revision 1
# baseline (speedup 1.0000x reference)
"""Trainium2 Bass kernel for nn_CoreBlock (circulant attention + 2-layer FFN).

Contract: kernel(**inputs) takes FULL unsharded inputs (as produced by
setup_inputs) and returns the FULL [16, 1024, 768] f32 output.

Strategy: pure data-parallel over batch — 8 NeuronCores x 2 batches each.
All weights replicated. Per core:
  phase A: LayerNorm(x) -> u, PE-transpose u -> u_dt, v = u_dt.T @ Wv' (per
           token-chunk), results gathered into a resident V tensor in SBUF.
  phase B: per head h: circulant matmul y[h] = C[h] @ v[h] using an 8-tile
           Toeplitz bank T[h,m] (m = (jc-ic) mod 8) precomputed on host;
           residual-added in place into X (X becomes x1 = x + y).
  phase C: 2x [Dense -> LayerNorm -> swish] with PE transposes between
           layers, then log_cosh(z + x1) = |w| + softplus(-2|w|) - log2.

Matmul operands are bf16 (full-rate PE, fp32 PSUM accumulation); stats and
elementwise math are fp32. LayerNorm rstd computations are batched into one
packed [128, NT] Sqrt per phase so the scalar engine does not thrash
activation-table loads.
"""

import math
import numpy as np
import ml_dtypes

import concourse.bass as bass
import concourse.tile as tile
from concourse import bacc, mybir
from concourse.bass_utils import run_bass_kernel_spmd

BF16 = ml_dtypes.bfloat16

B, N, D = 16, 1024, 768
H, HS, L = 12, 64, 2
EPS = 1e-6
NCORES = 8
BPC = B // NCORES          # batches per core
NJ = N // 128              # token chunks per batch (8)
NT = BPC * NJ              # token chunks per core (16)
DC = D // 128              # feature chunks (6)

F32 = mybir.dt.float32
BF = mybir.dt.bfloat16
Alu = mybir.AluOpType
Act = mybir.ActivationFunctionType

TRACE = False              # test harness sets this for profiling runs
TRACE_KW = {}
DEBUG_DUMPS = False

_cache = {}


def _build(cv_nonzero, bf_nonzero, lnf_uniform):
    """Construct the per-core Bass program. lnf_uniform: per-layer (cs, cb)
    if lnf scale/bias are uniform scalars, else None entries."""
    nc = bacc.Bacc("TRN2", target_bir_lowering=False, debug=False)

    xs = nc.dram_tensor("xs", (BPC, N, D), F32, kind="ExternalInput").ap()
    wv = nc.dram_tensor("wv", (D, D), BF, kind="ExternalInput").ap()
    wf = nc.dram_tensor("wf", (L, D, D), BF, kind="ExternalInput").ap()
    tb_d = nc.dram_tensor("tbank", (H, 128, NJ * 128), BF, kind="ExternalInput").ap()
    id32 = nc.dram_tensor("id32", (128, 128), F32, kind="ExternalInput").ap()
    idbf = nc.dram_tensor("idbf", (128, 128), BF, kind="ExternalInput").ap()
    cv_d = nc.dram_tensor("cv", (D,), F32, kind="ExternalInput").ap()
    bf_d = nc.dram_tensor("bfb", (L, D), F32, kind="ExternalInput").ap()
    lnfs_d = nc.dram_tensor("lnfs", (L, D), F32, kind="ExternalInput").ap()
    lnfb_d = nc.dram_tensor("lnfb", (L, D), F32, kind="ExternalInput").ap()
    out_d = nc.dram_tensor("out", (BPC, N, D), F32, kind="ExternalOutput").ap()
    dbg = None
    if DEBUG_DUMPS:
        dbg = {
            "dbg_u": nc.dram_tensor("dbg_u", (NT, 128, D), BF, kind="ExternalOutput").ap(),
            "dbg_v": nc.dram_tensor("dbg_v", (128, H, NJ, BPC, HS), BF, kind="ExternalOutput").ap(),
            "dbg_x1": nc.dram_tensor("dbg_x1", (128, BPC, NJ, D), F32, kind="ExternalOutput").ap(),
            "dbg_y1": nc.dram_tensor("dbg_y1", (NT, 128, D), BF, kind="ExternalOutput").ap(),
            "dbg_z1": nc.dram_tensor("dbg_z1", (NT, 128, D), BF, kind="ExternalOutput").ap(),
        }

    with tile.TileContext(nc) as tc:
        _emit(nc, tc, xs, wv, wf, tb_d, id32, idbf, cv_d, bf_d, lnfs_d, lnfb_d,
              out_d, cv_nonzero, bf_nonzero, lnf_uniform, dbg)
    nc.compile()
    return nc


def _emit(nc, tc, xs, wv, wf, tb_d, id32, idbf, cv_d, bf_d, lnfs_d, lnfb_d,
          out_d, cv_nonzero, bf_nonzero, lnf_uniform, dbg=None):
    from contextlib import ExitStack
    ctx = ExitStack()
    with ctx:
        consts = ctx.enter_context(tc.tile_pool(name="consts", bufs=1))
        xpool = ctx.enter_context(tc.tile_pool(name="xpool", bufs=1))
        vpool = ctx.enter_context(tc.tile_pool(name="vpool", bufs=1))
        acts = ctx.enter_context(tc.tile_pool(name="acts", bufs=18))
        tbp = ctx.enter_context(tc.tile_pool(name="tbp", bufs=2))
        upool = ctx.enter_context(tc.tile_pool(name="upool", bufs=3))
        dtp = ctx.enter_context(tc.tile_pool(name="dtp", bufs=3))
        stat = ctx.enter_context(tc.tile_pool(name="stat", bufs=4))
        statp = ctx.enter_context(tc.tile_pool(name="statp", bufs=2))
        wkp = ctx.enter_context(tc.tile_pool(name="wkp", bufs=3))
        outp = ctx.enter_context(tc.tile_pool(name="outp", bufs=3))
        ps_tr = ctx.enter_context(tc.tile_pool(name="ps_tr", bufs=2, space="PSUM"))
        ps_mm = ctx.enter_context(tc.tile_pool(name="ps_mm", bufs=2, space="PSUM"))

        # ---- constants ----
        wv_s = consts.tile([128, DC, D], BF, tag="wv")
        wf_s = consts.tile([128, L, DC, D], BF, tag="wf")
        i32 = consts.tile([128, 128], F32, tag="i32")
        ibf = consts.tile([128, 128], BF, tag="ibf")
        epst = consts.tile([128, 1], F32, tag="eps")
        nc.vector.memset(epst[:], EPS)
        zerot = consts.tile([128, 1], F32, tag="zero")
        nc.vector.memset(zerot[:], 0.0)
        onet = consts.tile([128, 1], F32, tag="one")
        nc.vector.memset(onet[:], 1.0)
        cvt = None
        if cv_nonzero:
            cvt = consts.tile([128, D], F32, tag="cv")
            nc.sync.dma_start(cvt[:], cv_d.to_broadcast((128, D)))
        bft = [None] * L
        lnfst = [None] * L
        lnfbt = [None] * L
        for l in range(L):
            if bf_nonzero[l]:
                bft[l] = consts.tile([128, D], F32, tag=f"bf{l}")
                nc.sync.dma_start(bft[l][:], bf_d[l].to_broadcast((128, D)))
            if lnf_uniform[l] is None:
                lnfst[l] = consts.tile([128, D], F32, tag=f"lnfs{l}")
                nc.sync.dma_start(lnfst[l][:], lnfs_d[l].to_broadcast((128, D)))
                lnfbt[l] = consts.tile([128, D], F32, tag=f"lnfb{l}")
                nc.sync.dma_start(lnfbt[l][:], lnfb_d[l].to_broadcast((128, D)))

        # ---- resident tensors ----
        X = xpool.tile([128, BPC, NJ, D], F32, tag="X")         # x, then x1
        V = vpool.tile([128, H, NJ, BPC, HS], BF, tag="V")      # per-head values

        # ================= phase A: LN + v-projection =================
        # Sub-batches of 4 chunks: stats -> packed sqrt/rstd -> normalize +
        # transpose + project, so the PE gets work early while later
        # chunks' stats are still streaming. After each batch's 8 chunks,
        # phase B (circulant + residual) for that batch is emitted so the
        # whole pipeline runs at half-kernel depth.
        mvA = statp.tile([128, NT, 2], F32, tag="mvA")
        sdA = statp.tile([128, NT], F32, tag="sdA")
        rsA = statp.tile([128, NT], F32, tag="rsA")
        AB = 4
        for t0 in range(0, NT, AB):
            for t in range(t0, t0 + AB):
                b, jc = divmod(t, NJ)
                xt = X[:, b, jc, :]
                nc.sync.dma_start(xt, xs[b, jc * 128:(jc + 1) * 128, :])
                st = stat.tile([128, 3, 6], F32, tag="bst")
                for g in range(3):
                    nc.vector.bn_stats(st[:, g, :], xt[:, g * 256:(g + 1) * 256])
                nc.vector.bn_aggr(mvA[:, t, :], st[:])
            nc.scalar.activation(sdA[:, t0:t0 + AB], mvA[:, t0:t0 + AB, 1],
                                 Act.Sqrt, bias=epst[:])
            nc.vector.reciprocal(rsA[:, t0:t0 + AB], sdA[:, t0:t0 + AB])
            if t0 == 0:
                # weights wanted shortly after the first chunks of x
                nc.sync.dma_start(wv_s[:], wv.rearrange("(c p) f -> p c f", p=128))
                nc.sync.dma_start(ibf[:], idbf)
                nc.sync.dma_start(i32[:], id32)
            for t in range(t0, t0 + AB):
                b, jc = divmod(t, NJ)
                xt = X[:, b, jc, :]
                u = upool.tile([128, D], BF, tag="u")
                nc.vector.tensor_scalar(u[:], xt, mvA[:, t, 0:1], rsA[:, t:t + 1],
                                        op0=Alu.subtract, op1=Alu.mult)
                if dbg is not None:
                    nc.sync.dma_start(dbg["dbg_u"][t], u[:])
                ptr = ps_tr.tile([128, D], BF, tag="tr")
                for c in range(DC):
                    nc.tensor.transpose(ptr[:, c * 128:(c + 1) * 128],
                                        u[:, c * 128:(c + 1) * 128], ibf[:])
                udt = dtp.tile([128, D], BF, tag="udt")
                nc.scalar.copy(udt[:], ptr[:])
                pv = ps_mm.tile([128, D], F32, tag="mm")
                for c in range(DC):
                    nc.tensor.matmul(pv[:, 0:512], udt[:, c * 128:(c + 1) * 128],
                                     wv_s[:, c, 0:512],
                                     start=(c == 0), stop=(c == DC - 1))
                    nc.tensor.matmul(pv[:, 512:D], udt[:, c * 128:(c + 1) * 128],
                                     wv_s[:, c, 512:D],
                                     start=(c == 0), stop=(c == DC - 1))
                vdst = V[:, :, jc, b, :]                             # [128, H, HS]
                pv3 = pv[:].rearrange("p (h k) -> p h k", h=H)
                if cv_nonzero:
                    cv3 = cvt[:].rearrange("p (h k) -> p h k", h=H)
                    nc.vector.tensor_tensor(vdst, pv3, cv3, op=Alu.add)
                else:
                    nc.vector.tensor_copy(vdst, pv3)
            if t0 // AB == 1:
                nc.sync.dma_start(wf_s[:], wf.rearrange("l (c p) f -> p l c f", p=128))
            # ======== phase B for batch bb once its 8 chunks are in V ======
            if (t0 + AB) % NJ == 0:
                bb = (t0 + AB) // NJ - 1
                for h in range(H):
                    tb = tbp.tile([128, NJ, 128], BF, tag="tb")
                    nc.sync.dma_start(tb[:], tb_d[h].rearrange("p (m f) -> p m f", m=NJ))
                    pc = ps_mm.tile([128, NJ, HS], F32, tag="mm")
                    for m in range(NJ):
                        for ic in range(NJ):
                            jc = (ic + m) % NJ
                            nc.tensor.matmul(pc[:, ic, :], tb[:, m, :],
                                             V[:, h, jc, bb, :],
                                             start=(m == 0 and ic == 0),
                                             stop=(m == NJ - 1),
                                             skip_group_check=True)
                    xap = X[:, bb, :, h * HS:(h + 1) * HS]           # [128,NJ,HS]
                    nc.vector.tensor_tensor(xap, xap, pc[:], op=Alu.add)

        if dbg is not None:
            nc.sync.dma_start(dbg["dbg_v"][:], V[:])
            nc.sync.dma_start(dbg["dbg_x1"][:], X[:])

        # ================= phase C: FFN x2 + log_cosh =================
        inv_d = 1.0 / D
        zcur = [None] * NT
        for l in range(L):
            fast = lnf_uniform[l] is not None
            sums = statp.tile([128, NT], F32, tag=f"sum{l}")
            ssq = statp.tile([128, NT], F32, tag=f"ssq{l}")
            for t in range(NT):
                b, jc = divmod(t, NJ)
                src = X[:, b, jc, :] if l == 0 else zcur[t][:]
                ptr = ps_tr.tile([128, D], F32 if l == 0 else BF, tag="tr")
                ident = i32 if l == 0 else ibf
                for c in range(DC):
                    nc.tensor.transpose(ptr[:, c * 128:(c + 1) * 128],
                                        src[:, c * 128:(c + 1) * 128], ident[:])
                zdt = dtp.tile([128, D], BF, tag="zdt")
                nc.scalar.copy(zdt[:], ptr[:])
                pf = ps_mm.tile([128, D], F32, tag="mm")
                for c in range(DC):
                    nc.tensor.matmul(pf[:, 0:512], zdt[:, c * 128:(c + 1) * 128],
                                     wf_s[:, l, c, 0:512],
                                     start=(c == 0), stop=(c == DC - 1))
                    nc.tensor.matmul(pf[:, 512:D], zdt[:, c * 128:(c + 1) * 128],
                                     wf_s[:, l, c, 512:D],
                                     start=(c == 0), stop=(c == DC - 1))
                if bf_nonzero[l]:
                    nc.vector.tensor_tensor(pf[:], pf[:], bft[l][:], op=Alu.add)
                # mean via the ACT copy's accumulator output; sumsq on DVE
                # from the bf16 copy (PSUM allows only one non-scalar input,
                # and bf16 SBUF reads run in 2x mode anyway).
                y = acts.tile([128, D], BF, tag="acts")
                nc.scalar.activation(y[:], pf[:], Act.Copy,
                                     accum_out=sums[:, t:t + 1])
                scr = wkp.tile([128, D], BF, tag="scr")
                nc.vector.scalar_tensor_tensor(
                    scr[:], y[:], 0.0, y[:], op0=Alu.add, op1=Alu.mult,
                    accum_out=ssq[:, t:t + 1])
                if dbg is not None and l == 0:
                    nc.sync.dma_start(dbg["dbg_y1"][t], y[:])
                zcur[t] = y
            # batched LN epilogue: var = E[y^2] - mu^2, one Sqrt, one recip
            muA = statp.tile([128, NT], F32, tag=f"mu{l}")
            nc.vector.tensor_scalar(muA[:], sums[:], inv_d, None, op0=Alu.mult)
            m2A = statp.tile([128, NT], F32, tag=f"m2{l}")
            nc.vector.tensor_scalar(m2A[:], ssq[:], inv_d, None, op0=Alu.mult)
            varA = statp.tile([128, NT], F32, tag=f"var{l}")
            nc.vector.scalar_tensor_tensor(varA[:], muA[:], -1.0, muA[:],
                                           op0=Alu.mult, op1=Alu.mult)
            nc.vector.tensor_tensor(varA[:], m2A[:], varA[:], op=Alu.add)
            sdF = statp.tile([128, NT], F32, tag=f"sd{l}")
            nc.scalar.activation(sdF[:], varA[:], Act.Sqrt, bias=epst[:])
            rsF = statp.tile([128, NT], F32, tag=f"rs{l}")
            nc.vector.reciprocal(rsF[:], sdF[:])
            biasF = statp.tile([128, NT], F32, tag=f"bi{l}")
            if fast:
                cs, cb = lnf_uniform[l]
                if cs != 1.0:
                    nc.vector.tensor_scalar(rsF[:], rsF[:], float(cs), None,
                                            op0=Alu.mult)
                nc.vector.scalar_tensor_tensor(biasF[:], muA[:], -1.0, rsF[:],
                                               op0=Alu.mult, op1=Alu.mult)
                if cb != 0.0:
                    nc.vector.tensor_scalar(biasF[:], biasF[:], float(cb), None,
                                            op0=Alu.add)
                for t in range(NT):
                    y = zcur[t]
                    nc.scalar.activation(y[:], y[:], Act.Silu,
                                         bias=biasF[:, t:t + 1],
                                         scale=rsF[:, t:t + 1])
            else:
                for t in range(NT):
                    y = zcur[t]
                    tmp = acts.tile([128, D], BF, tag="acts")
                    nc.vector.tensor_scalar(tmp[:], y[:], muA[:, t:t + 1],
                                            rsF[:, t:t + 1],
                                            op0=Alu.subtract, op1=Alu.mult)
                    nc.vector.tensor_tensor(tmp[:], tmp[:], lnfst[l][:],
                                            op=Alu.mult)
                    nc.vector.tensor_tensor(tmp[:], tmp[:], lnfbt[l][:],
                                            op=Alu.add)
                    nc.scalar.activation(tmp[:], tmp[:], Act.Silu,
                                         bias=zerot[:])
                    zcur[t] = tmp

        if dbg is not None:
            for t in range(NT):
                nc.sync.dma_start(dbg["dbg_z1"][t], zcur[t][:])

        # tail: log_cosh(w) = |w| + log1p(exp(-2|w|)) - log2.
        # |w| is computed in place over X (x1 is dead afterwards) so it stays
        # fp32 with no extra SBUF; exp and log1p batch on the scalar engine
        # (one table load each) with a bf16 intermediate that reuses V's slot.
        ln2 = math.log(2.0)
        awl = vpool.tile([128, NT, D], BF, tag="V")
        for t in range(NT):
            b, jc = divmod(t, NJ)
            xt = X[:, b, jc, :]
            nc.vector.tensor_tensor(xt, xt, zcur[t][:], op=Alu.add)
            nc.vector.scalar_tensor_tensor(xt, xt, -1.0, xt,
                                           op0=Alu.mult, op1=Alu.max)
        for t in range(NT):
            b, jc = divmod(t, NJ)
            nc.scalar.activation(awl[:, t, :], X[:, b, jc, :], Act.Exp,
                                 bias=zerot[:], scale=-2.0)
        for t in range(NT):
            nc.scalar.activation(awl[:, t, :], awl[:, t, :], Act.Ln,
                                 bias=onet[:], scale=1.0)
        for t in range(NT):
            b, jc = divmod(t, NJ)
            ot = outp.tile([128, D], F32, tag="ot")
            nc.vector.scalar_tensor_tensor(ot[:], X[:, b, jc, :], -ln2,
                                           awl[:, t, :], op0=Alu.add,
                                           op1=Alu.add)
            nc.sync.dma_start(out_d[b, jc * 128:(jc + 1) * 128, :], ot[:])


def _prep(inputs):
    x = np.asarray(inputs["x"], np.float32)
    ln1_s = np.asarray(inputs["ln1_scale"], np.float32)
    ln1_b = np.asarray(inputs["ln1_bias"], np.float32)
    Wv = np.asarray(inputs["Wv"], np.float32)
    alpha = np.asarray(inputs["alpha"], np.float32)
    Wf = np.asarray(inputs["Wf"], np.float32)
    bfv = np.asarray(inputs["bf"], np.float32)
    lnf_s = np.asarray(inputs["lnf_scale"], np.float32)
    lnf_b = np.asarray(inputs["lnf_bias"], np.float32)

    Wv_flat = Wv.transpose(1, 0, 2).reshape(D, H * HS)
    Wvp = (ln1_s[:, None] * Wv_flat).astype(BF16)
    cv = (ln1_b @ Wv_flat).astype(np.float32)

    ar = alpha[:, (-np.arange(N)) % N]
    ar2 = np.concatenate([ar, ar], axis=1)
    m_ = np.arange(NJ)[:, None, None]
    p_ = np.arange(128)[None, :, None]
    f_ = np.arange(128)[None, None, :]
    T = ar2[:, N + 128 * m_ + p_ - f_]                  # [H, NJ, 128, 128]
    tbank = np.ascontiguousarray(
        T.transpose(0, 2, 1, 3).reshape(H, 128, NJ * 128)).astype(BF16)

    cv_nonzero = bool(np.any(cv))
    bf_nonzero = tuple(bool(np.any(bfv[l])) for l in range(L))
    lnf_uniform = []
    for l in range(L):
        s, bb = lnf_s[l], lnf_b[l]
        if np.all(s == s[0]) and np.all(bb == bb[0]):
            lnf_uniform.append((float(s[0]), float(bb[0])))
        else:
            lnf_uniform.append(None)
    key = (cv_nonzero, bf_nonzero, tuple(lnf_uniform), DEBUG_DUMPS)

    common = {
        "wv": np.ascontiguousarray(Wvp),
        "wf": Wf.astype(BF16),
        "tbank": tbank,
        "id32": np.eye(128, dtype=np.float32),
        "idbf": np.eye(128, dtype=BF16),
        "cv": cv,
        "bfb": bfv,
        "lnfs": lnf_s,
        "lnfb": lnf_b,
    }
    return x, key, common, (cv_nonzero, bf_nonzero, lnf_uniform)


def kernel(**inputs):
    x, key, common, flags = _prep(inputs)
    if key not in _cache:
        _cache[key] = _build(*flags)
    nc = _cache[key]
    in_maps = []
    for i in range(NCORES):
        m = dict(common)
        m["xs"] = np.ascontiguousarray(x[i * BPC:(i + 1) * BPC])
        in_maps.append(m)
    res = run_bass_kernel_spmd(nc, in_maps, core_ids=list(range(NCORES)),
                               trace=TRACE, **TRACE_KW)
    kernel.last_result = res
    out = np.empty((B, N, D), np.float32)
    for i in range(NCORES):
        out[i * BPC:(i + 1) * BPC] = res.results[i]["out"]
    return out

